# revision 4
# baseline (speedup 1.0000x reference)
"""Trainium2 Bass kernel: GAT-style attention layer, data-parallel over 8 NeuronCores.

Reference computation (per node n, K=32 neighbors, D=128 features, L=64 labels):
    h     = lrelu(x @ W)                  [N,K,D]
    e     = lrelu(h @ v + bias)           [N,K,1]
    alpha = softmax_k(e)                  [N,K]
    out   = sum_k alpha[n,k] * labels[n,k,:]   [N,L]

Sharding: pure data parallel over nodes (6250/core, zero-padded to 6400).

Key structure (v2): everything that contracts over a 128-partition axis rides
the PE with the LARGE tensor as the *stationary* operand and a single-column
moving operand, so PE engine time ~ moving columns only:
  mm1    z^T[e,(k,n)] = W^T @ x^T        16x512-col fp8 matmuls / 256-node tile
  relu   h = relu(z) PSUM->SBUF bf16, split across ScalarE+VectorE chunks
         (the 0.2z part of lrelu is restored exactly inside the score matmul)
  score  s[n, 32s+k] = 0.8 v^T h_k + 0.2 (Wv)^T x_k  -- per (k,sub): two
         1-column matmuls with h/x slices as stationary  [128n x 64] PSUM
  e/exp  ACT Prelu + Exp on [128,64]; per-sub row sums on gpsimd
  alphaT PE transpose of raw exp-weights -> [32k, 256n]
  agg    out^T[l, n] = lab[n]^T @ wT[:, n]: one 1-column matmul per node with
         the node's fp8 label block [32,64] as stationary
  norm   softmax 1/sum applied on the HOST (sums DMA'd out, 1KB/tile)

Quantization: x fp8-e3m4, labels fp8-e3m4, weights bf16, out bf16.
Per-core DRAM traffic ~39MB; DMA floor ~4.5us per 256-node tile.
"""
import sys

sys.path.insert(0, "/opt/trn_rl_repo")
import numpy as np

N, K, D, L = 50000, 32, 128, 64
NEG = 0.2
NCORES = 8
NPER = N // NCORES          # 6250
TN = 256                    # nodes per tile
NSUB = TN // 128            # sub-tiles of 128 nodes
NPAD = 6400                 # padded nodes per core
NT = NPAD // TN             # 25 tiles
NCH = 8                     # mm1 chunks per tile (1024 cols each)

# relu-chunk engine schedule per tile: 'A' = ScalarE(ACT), 'D' = VectorE(DVE).
# ACT is the faster PSUM->SBUF mover but also owns Prelu/Exp; target ~4.25
# ACT chunks per tile on average.
SCHEDULES = [
    "ADADADAD",   # 4 ACT / 4 DVE
    "ADADADAD",
    "ADADADAD",
    "ADADADAA",   # 6 ACT / 2 DVE every 4th tile -> avg 4.5
]

LAST_RESULT = None
_cache = {}


def build(with_bias):
    import concourse.bass as bass
    import concourse.tile as tile
    from concourse import bacc, mybir

    f32 = mybir.dt.float32
    bf16 = mybir.dt.bfloat16
    f8 = mybir.dt.float8e3
    AF = mybir.ActivationFunctionType
    OP = mybir.AluOpType
    PSUM = bass.MemorySpace.PSUM
    nt = NT

    nc = bacc.Bacc(
        "TRN2", target_bir_lowering=False, debug=False, num_devices=NCORES
    )
    x_ext = nc.declare_dram_parameter("x", [nt, 128, K * TN], f8, False)
    lab_ext = nc.declare_dram_parameter("lab", [nt, 32, TN * L], f8, False)
    w_ext = nc.declare_dram_parameter("w", [D, D], bf16, False)
    v08_ext = nc.declare_dram_parameter("v08", [D, 1], bf16, False)
    wv02_ext = nc.declare_dram_parameter("wv02", [D, 1], bf16, False)
    b64_ext = nc.declare_dram_parameter("b64", [128, 64], f32, False)
    out_ext = nc.declare_dram_parameter("out", [nt, L, TN], bf16, isOutput=True)
    sums_ext = nc.declare_dram_parameter("sums", [nt, 128, NSUB], f32, isOutput=True)

    with tile.TileContext(nc) as tc:
        with (
            tc.tile_pool(name="const", bufs=1) as const,
            tc.tile_pool(name="xp", bufs=3) as xp,
            tc.tile_pool(name="labp", bufs=2) as labp,
            tc.tile_pool(name="hp", bufs=2) as hp,
            tc.tile_pool(name="smallp", bufs=2) as smallp,
            tc.tile_pool(name="outp", bufs=2) as outp,
            tc.tile_pool(name="zps", bufs=2, space=PSUM) as zps,
            tc.tile_pool(name="sps", bufs=2, space=PSUM) as sps,
            tc.tile_pool(name="atps", bufs=1, space=PSUM) as atps,
            tc.tile_pool(name="ops", bufs=1, space=PSUM) as ops,
        ):
            W_sb = const.tile([128, 128], bf16)
            nc.sync.dma_start(W_sb[:], w_ext[:])
            v08 = const.tile([128, 1], bf16)
            nc.sync.dma_start(v08[:], v08_ext[:])
            wv02 = const.tile([128, 1], bf16)
            nc.sync.dma_start(wv02[:], wv02_ext[:])
            if with_bias:
                b64 = const.tile([128, 64], f32)
                nc.sync.dma_start(b64[:], b64_ext[:])
            ones = const.tile([128, 128], bf16)
            nc.vector.memset(ones[:], 1.0)
            ident = const.tile([128, 128], bf16)         # identity matrix
            nc.gpsimd.affine_select(
                ident[:], ones[:], pattern=[[1, 128]],
                compare_op=OP.is_equal, fill=0.0, base=0, channel_multiplier=-1,
            )

            # PE warmup burst: dummy matmuls (deps ready ~1us in) while the
            # first x tile loads; the p-state model needs ~3.4us of early PE
            # activity to reach the full 2.4GHz clock.
            warm_ps = zps.tile([128, 1024], f32, name="warm_ps", tag="z")
            for _ in range(32):
                nc.tensor.matmul(
                    warm_ps[:, 0:128], W_sb[:], ones[:], skip_group_check=True
                )

            prev = None   # state of tile t-1 awaiting its score/agg tail

            def emit_head(st):
                """e = lrelu(s) (+bias), w = exp(e), per-sub row sums."""
                s_ps = st["s_ps"]
                e_sb = smallp.tile([128, 64], f32, name="e_sb", tag="e_sb")
                if with_bias:
                    # bias varies along the free (k) axis: add the host-
                    # replicated [128,64] bias tile, then Prelu from SBUF
                    sb = smallp.tile([128, 64], f32, name="sb", tag="sb")
                    nc.vector.tensor_tensor(sb[:], s_ps[:], b64[:], OP.add)
                    nc.scalar.activation(e_sb[:], sb[:], AF.Prelu, alpha=NEG)
                else:
                    nc.scalar.activation(e_sb[:], s_ps[:], AF.Prelu, alpha=NEG)
                w_sb = smallp.tile([128, 64], bf16, name="w_sb", tag="w_sb")
                nc.scalar.activation(w_sb[:], e_sb[:], AF.Exp)
                sums = smallp.tile([128, NSUB], f32, name="sums", tag="sums")
                nc.vector.tensor_reduce(
                    sums[:], w_sb[:].rearrange("p (s k) -> p s k", s=NSUB),
                    op=OP.add, axis=mybir.AxisListType.X,
                )
                st["w_sb"], st["sums"] = w_sb, sums

            def emit_transp(st):
                """Raw exp-weights -> [32k, 256n] via PE transposes."""
                aT_ps = atps.tile([32, 256], bf16, name="aT_ps", tag="aT_ps")
                for s in range(NSUB):
                    nc.tensor.transpose(
                        aT_ps[:, 128 * s:128 * (s + 1)],
                        st["w_sb"][:, 32 * s:32 * (s + 1)], ident[:],
                    )
                aT_sb = smallp.tile([32, 256], bf16, name="aT_sb", tag="aT_sb")
                nc.vector.tensor_copy(aT_sb[:], aT_ps[:])
                st["aT_sb"] = aT_sb
                nc.sync.dma_start(sums_ext[st["t"]], st["sums"][:])

            def emit_agg_alloc(st):
                st["o_ps"] = ops.tile([64, TN], f32, name="o_ps", tag="o_ps")

            def emit_agg(st, q):
                """Aggregation for nodes [64q, 64q+64): one 1-col matmul per
                node with its fp8 label block [32,64] as stationary."""
                lab_sb, aT_sb, o_ps = st["lab_sb"], st["aT_sb"], st["o_ps"]
                for n in range(64 * q, 64 * q + 64):
                    nc.tensor.matmul(
                        o_ps[:, n:n + 1],
                        lab_sb[:, L * n:L * (n + 1)],
                        aT_sb[:, n:n + 1],
                    )

            def emit_out(st):
                o_sb = outp.tile([64, TN], bf16, name="o_sb", tag="o_sb")
                nc.vector.tensor_copy(o_sb[:], st["o_ps"][:])
                nc.sync.dma_start(out_ext[st["t"]], o_sb[:])

            def drain(st):
                emit_head(st)
                emit_transp(st)
                emit_agg_alloc(st)
                for q in range(4):
                    emit_agg(st, q)
                emit_out(st)

            for t in range(nt):
                sched = SCHEDULES[t % len(SCHEDULES)]
                x_sb = xp.tile([128, K * TN], f8)
                if t == 0:
                    # quarter the first x load so chunk 0's matmuls start
                    # after ~256KB instead of a full 1MB
                    qn = K * TN // 4
                    for qi in range(4):
                        nc.sync.dma_start(
                            x_sb[:, qi * qn:(qi + 1) * qn],
                            x_ext[t][:, qi * qn:(qi + 1) * qn],
                        )
                else:
                    nc.sync.dma_start(x_sb[:], x_ext[t][:])
                lab_sb = labp.tile([32, TN * L], f8)
                nc.sync.dma_start(lab_sb[:], lab_ext[t][:])

                h_sb = hp.tile([128, K * TN], bf16)
                s_ps = sps.tile([128, 64], f32, name="s_ps", tag="sps")

                def emit_scores(c):
                    # chunk c covers k in 4c..4c+3; per (k, sub): column
                    # 32*sub + k of s_ps accumulates
                    #   0.8 * v^T relu(z)  +  0.2 * (Wv)^T x
                    # (exact lrelu: lrelu(z) = 0.8 relu(z) + 0.2 z)
                    for k in range(4 * c, 4 * c + 4):
                        for s in range(NSUB):
                            col = 32 * s + k
                            base = k * TN + s * 128
                            nc.tensor.matmul(
                                s_ps[:, col:col + 1],
                                h_sb[:, base:base + 128], v08[:],
                                start=True, stop=False,
                            )
                            nc.tensor.matmul(
                                s_ps[:, col:col + 1],
                                x_sb[:, base:base + 128], wv02[:],
                                start=False, stop=True,
                            )

                for c in range(NCH):
                    z_ps = zps.tile([128, 1024], f32, name="z_ps", tag="z")
                    nc.tensor.matmul(
                        z_ps[:, 0:512], W_sb[:], x_sb[:, c * 1024:c * 1024 + 512]
                    )
                    nc.tensor.matmul(
                        z_ps[:, 512:1024], W_sb[:],
                        x_sb[:, c * 1024 + 512:(c + 1) * 1024],
                    )
                    hc = h_sb[:, c * 1024:(c + 1) * 1024]
                    if sched[c] == "A":
                        nc.scalar.activation(hc, z_ps[:], AF.Relu)
                    else:
                        nc.vector.tensor_scalar_max(hc, z_ps[:], 0.0)
                    if c == 0 and prev is not None:
                        emit_head(prev)
                    if c == 1 and prev is not None:
                        emit_transp(prev)
                        emit_agg_alloc(prev)
                    if c >= 1:
                        emit_scores(c - 1)
                    if c in (2, 3, 4, 5) and prev is not None:
                        emit_agg(prev, c - 2)
                    if c == 6 and prev is not None:
                        emit_out(prev)
                emit_scores(NCH - 1)

                prev = {"t": t, "s_ps": s_ps, "lab_sb": lab_sb}

            drain(prev)
    nc.compile()
    return nc


def shard_x(x, nt=NT, nper=NPER):
    import ml_dtypes

    f8 = ml_dtypes.float8_e3m4
    xs = np.zeros((nt * TN, K, D), f8)
    xs[:nper] = x.astype(f8)
    # [t, n, k, d] -> [t, d, k, n] -> col = k*TN + n
    return np.ascontiguousarray(
        xs.reshape(nt, TN, K, D).transpose(0, 3, 2, 1)
    ).reshape(nt, 128, K * TN)


def shard_lab(lab, nt=NT, nper=NPER):
    import ml_dtypes

    f8 = ml_dtypes.float8_e3m4
    ls = np.zeros((nt * TN, K, L), f8)
    ls[:nper] = lab.astype(f8)
    # [t, n, k, l] -> [t, k, n, l] -> rows k, col = n*L + l
    return np.ascontiguousarray(
        ls.reshape(nt, TN, K, L).transpose(0, 2, 1, 3)
    ).reshape(nt, 32, TN * L)


def make_in_maps(inputs):
    import ml_dtypes

    bf16 = ml_dtypes.bfloat16
    x = np.asarray(inputs["para_neighbors"], np.float32)
    lab = np.asarray(inputs["para_nei_labels"], np.float32)
    Wm = np.ascontiguousarray(np.asarray(inputs["linear"], np.float32))
    v = np.ascontiguousarray(np.asarray(inputs["e_vec"], np.float32))
    b = np.asarray(inputs["bias"], np.float32).reshape(K)

    Wb = Wm.astype(bf16).astype(np.float32)
    vb = v.astype(bf16).astype(np.float32)
    W16 = np.ascontiguousarray(Wm.astype(bf16))
    v08 = np.ascontiguousarray((0.8 * vb).astype(bf16))
    # 0.2*(W@v) from the bf16-rounded W/v so the correction matches the PE's z
    wv02 = np.ascontiguousarray((NEG * (Wb @ vb)).astype(bf16))
    # b64[p, 32s+k] = bias[k] (same for every partition row)
    b64 = np.ascontiguousarray(
        np.tile(np.concatenate([b, b])[None, :], (128, 1))
    ).astype(np.float32)

    in_maps = []
    for i in range(NCORES):
        xf = shard_x(x[i * NPER:(i + 1) * NPER])
        lf = shard_lab(lab[i * NPER:(i + 1) * NPER])
        in_maps.append(
            {"x": xf, "lab": lf, "w": W16, "v08": v08, "wv02": wv02, "b64": b64}
        )
    return in_maps


def unshard_output(res_i):
    # out[t, l, c] = raw_sum for node n = t*TN + c; sums[t, p, s] for
    # node n = t*TN + s*128 + p. Softmax normalization happens here.
    o = np.asarray(res_i["out"]).astype(np.float32)     # [nt, L, TN]
    sums = np.asarray(res_i["sums"]).astype(np.float32)  # [nt, 128, NSUB]
    raw = o.transpose(0, 2, 1).reshape(NT * TN, L)
    s = sums.transpose(0, 2, 1).reshape(NT * TN)
    return (raw[:NPER] / s[:NPER, None]).astype(np.float32)


def kernel(para_neighbors, para_nei_labels, linear, e_vec, bias):
    from concourse.bass_utils import run_bass_kernel_spmd

    global LAST_RESULT
    with_bias = bool(np.any(np.asarray(bias)))
    key = ("nc", with_bias)
    if key not in _cache:
        _cache[key] = build(with_bias)
        _cache["nc"] = _cache[key]
    nc = _cache[key]

    in_maps = make_in_maps({
        "para_neighbors": para_neighbors, "para_nei_labels": para_nei_labels,
        "linear": linear, "e_vec": e_vec, "bias": bias,
    })
    res = run_bass_kernel_spmd(nc, in_maps, core_ids=list(range(NCORES)))
    LAST_RESULT = res
    outs = [unshard_output(res.results[i]) for i in range(NCORES)]
    return np.ascontiguousarray(np.concatenate(outs, axis=0))


# revision 6
# speedup vs baseline: 1.4063x; 1.4063x over previous
"""Trainium2 Bass kernel: GAT-style attention layer, data-parallel over 8 NeuronCores.

Reference computation (per node n, K=32 neighbors, D=128 features, L=64 labels):
    h     = lrelu(x @ W)                  [N,K,D]
    e     = lrelu(h @ v + bias)           [N,K,1]
    alpha = softmax_k(e)                  [N,K]
    out   = sum_k alpha[n,k] * labels[n,k,:]   [N,L]

Sharding: pure data parallel over nodes (6250/core, zero-padded to 6400).

Structure (v3): every contraction over a 128-partition axis rides the PE with
the LARGE tensor as the *stationary* operand and a 1..4-column moving operand
(PE engine time ~ moving columns only):
  mm1    z^T[e,(k,n)] = W^T @ x^T      16x512-col fp8 matmuls / 256-node tile
  act    h = lrelu(z) (ScalarE chunks, full Prelu) or relu(z) (VectorE
         chunks; the 0.2z part is restored by an x-correction matmul)
  score  s[n, 32s+k]: per (k,sub) a 1-column matmul with the h slice as
         stationary (+ 0.2(Wv)^T x correction for VectorE chunks)
  e/exp  ACT Prelu + Exp on [128,64]
  alphaT 8 32x32 PE transposes place node-quarter j's exp-weights at
         partition block 32j of a persistent (startup-zeroed) PSUM tile;
         one DVE copy -> aT4 [128, 256n] with zeros off-block
  agg    out^T[l, 4 nodes] per matmul: stationary = 4 nodes' label blocks
         stacked [128=(4x32k), 64l] fp8; off-block zeros in aT4 kill the
         cross-node terms. 64 Ldweights+matmuls per tile.
  sums   gpsimd partition-reduce of aT4 -> [1, 256]; softmax 1/sum applied
         on the HOST (sums are DMA'd out, 1KB/tile)

Quantization: x fp8-e3m4, labels fp8-e3m4, weights bf16, out bf16.
Per-core DRAM traffic ~39MB; DMA floor ~4.5us per 256-node tile.
"""
import sys

sys.path.insert(0, "/opt/trn_rl_repo")
import numpy as np

N, K, D, L = 50000, 32, 128, 64
NEG = 0.2
NCORES = 8
NPER = N // NCORES          # 6250
TN = 256                    # nodes per tile
NSUB = TN // 128            # sub-tiles of 128 nodes
NPAD = 6400                 # padded nodes per core
NT = NPAD // TN             # 25 tiles
NCH = 8                     # mm1 chunks per tile (1024 cols each)

# relu-chunk engine schedule: 'A' = ScalarE (full Prelu), 'D' = VectorE
# (relu-only + PE x-correction). Chunk 0 must be 'D' so ACT starts each tile
# with the previous tile's Prelu/Exp.
SCHED = "DADADADA"

LAST_RESULT = None
_cache = {}


def build(with_bias):
    import concourse.bass as bass
    import concourse.tile as tile
    from concourse import bacc, mybir

    f32 = mybir.dt.float32
    bf16 = mybir.dt.bfloat16
    f8 = mybir.dt.float8e3
    AF = mybir.ActivationFunctionType
    OP = mybir.AluOpType
    PSUM = bass.MemorySpace.PSUM
    nt = NT

    nc = bacc.Bacc(
        "TRN2", target_bir_lowering=False, debug=False, num_devices=NCORES
    )
    x_ext = nc.declare_dram_parameter("x", [nt, 128, K * TN], f8, False)
    lab_ext = nc.declare_dram_parameter("lab", [nt, 128, 64 * L], f8, False)
    w_ext = nc.declare_dram_parameter("w", [D, D], bf16, False)
    v10_ext = nc.declare_dram_parameter("v10", [D, 1], bf16, False)
    v08_ext = nc.declare_dram_parameter("v08", [D, 1], bf16, False)
    wv02_ext = nc.declare_dram_parameter("wv02", [D, 1], bf16, False)
    b64_ext = nc.declare_dram_parameter("b64", [128, 64], f32, False)
    out_ext = nc.declare_dram_parameter("out", [nt, L, TN], bf16, isOutput=True)
    sums_ext = nc.declare_dram_parameter("sums", [nt, 1, TN], f32, isOutput=True)

    with tile.TileContext(nc) as tc:
        with (
            tc.tile_pool(name="const", bufs=1) as const,
            tc.tile_pool(name="xp", bufs=3) as xp,
            tc.tile_pool(name="labp", bufs=2) as labp,
            tc.tile_pool(name="hp", bufs=2) as hp,
            tc.tile_pool(name="smallp", bufs=2) as smallp,
            tc.tile_pool(name="outp", bufs=2) as outp,
            tc.tile_pool(name="zps", bufs=2, space=PSUM) as zps,
            tc.tile_pool(name="sps", bufs=2, space=PSUM) as sps,
            tc.tile_pool(name="atps", bufs=1, space=PSUM) as atps,
            tc.tile_pool(name="ops", bufs=1, space=PSUM) as ops,
        ):
            W_sb = const.tile([128, 128], bf16)
            nc.sync.dma_start(W_sb[:], w_ext[:])
            v10 = const.tile([128, 1], bf16)
            nc.sync.dma_start(v10[:], v10_ext[:])
            v08 = const.tile([128, 1], bf16)
            nc.sync.dma_start(v08[:], v08_ext[:])
            wv02 = const.tile([128, 1], bf16)
            nc.sync.dma_start(wv02[:], wv02_ext[:])
            if with_bias:
                b64 = const.tile([128, 64], f32)
                nc.sync.dma_start(b64[:], b64_ext[:])
            ones = const.tile([128, 128], bf16)
            nc.vector.memset(ones[:], 1.0)
            ident = const.tile([128, 128], bf16)         # identity matrix
            nc.gpsimd.affine_select(
                ident[:], ones[:], pattern=[[1, 128]],
                compare_op=OP.is_equal, fill=0.0, base=0, channel_multiplier=-1,
            )
            # persistent exp-weight transpose target: node-quarter j occupies
            # partition block 32j; everything off-block is zeroed ONCE here
            # and never written again, so cross-node terms in the batched agg
            # matmul multiply against exact zeros.
            aT4_ps = atps.tile([128, TN], bf16, name="aT4_ps", tag="aT4_ps")
            nc.vector.memset(aT4_ps[:], 0.0)

            # PE warmup burst: dummy matmuls (deps ready ~1us in) while the
            # first x tile loads; the p-state model needs ~3.4us of early PE
            # activity to reach the full 2.4GHz clock.
            warm_ps = zps.tile([128, 1024], f32, name="warm_ps", tag="z")
            for _ in range(32):
                nc.tensor.matmul(
                    warm_ps[:, 0:128], W_sb[:], ones[:], skip_group_check=True
                )

            prev = None   # state of tile t-1 awaiting its score/agg tail

            def emit_head(st):
                """e = lrelu(s) (+bias), w = exp(e). First ACT ops of a tile."""
                s_ps = st["s_ps"]
                e_sb = smallp.tile([128, 64], f32, name="e_sb", tag="e_sb")
                if with_bias:
                    sb = smallp.tile([128, 64], f32, name="sb", tag="sb")
                    nc.vector.tensor_tensor(sb[:], s_ps[:], b64[:], OP.add)
                    nc.scalar.activation(e_sb[:], sb[:], AF.Prelu, alpha=NEG)
                else:
                    nc.scalar.activation(e_sb[:], s_ps[:], AF.Prelu, alpha=NEG)
                w_sb = smallp.tile([128, 64], bf16, name="w_sb", tag="w_sb")
                nc.scalar.activation(w_sb[:], e_sb[:], AF.Exp)
                st["w_sb"] = w_sb

            def emit_transp(st):
                """8 32x32 transposes: sub s node-quarter j -> aT4_ps rows
                [32j,32j+32), cols [128s+32j, +32). Then one DVE copy to SBUF
                (zeros off-block come along) + gpsimd per-node sums + DMA."""
                w_sb = st["w_sb"]
                for s in range(NSUB):
                    for j in range(4):
                        nc.tensor.transpose(
                            aT4_ps[32 * j:32 * j + 32,
                                   128 * s + 32 * j:128 * s + 32 * j + 32],
                            w_sb[32 * j:32 * j + 32, 32 * s:32 * s + 32],
                            ident[32 * j:32 * j + 32, 32 * j:32 * j + 32],
                            tile_position=(32 * j, 32 * j),
                        )
                aT4 = smallp.tile([128, TN], bf16, name="aT4", tag="aT4")
                nc.vector.tensor_copy(aT4[:], aT4_ps[:])
                st["aT4"] = aT4
                sums = smallp.tile([1, TN], f32, name="sums", tag="sums")
                nc.gpsimd.tensor_reduce(
                    sums[:], aT4[:], op=OP.add, axis=mybir.AxisListType.C,
                )
                nc.sync.dma_start(sums_ext[st["t"]], sums[:])
                st["o_ps"] = ops.tile([64, TN], f32, name="o_ps", tag="o_ps")

            def emit_agg(st, q):
                """Aggregation for node groups [16q, 16q+16): one 4-column
                matmul per group of 4 nodes; stationary = their label blocks
                stacked [128, 64] fp8."""
                lab_sb, aT4, o_ps = st["lab_sb"], st["aT4"], st["o_ps"]
                for g in range(16 * q, 16 * q + 16):
                    nc.tensor.matmul(
                        o_ps[:, 4 * g:4 * g + 4],
                        lab_sb[:, 64 * g:64 * (g + 1)],
                        aT4[:, 4 * g:4 * g + 4],
                    )

            def emit_out(st):
                o_sb = outp.tile([64, TN], bf16, name="o_sb", tag="o_sb")
                nc.scalar.activation(o_sb[:], st["o_ps"][:], AF.Copy)
                nc.sync.dma_start(out_ext[st["t"]], o_sb[:])

            for t in range(nt):
                x_sb = xp.tile([128, K * TN], f8)
                if t == 0:
                    # quarter the first x load so chunk 0's matmuls start
                    # after ~256KB instead of a full 1MB
                    qn = K * TN // 4
                    for qi in range(4):
                        nc.sync.dma_start(
                            x_sb[:, qi * qn:(qi + 1) * qn],
                            x_ext[t][:, qi * qn:(qi + 1) * qn],
                        )
                else:
                    nc.sync.dma_start(x_sb[:], x_ext[t][:])
                lab_sb = labp.tile([128, 64 * L], f8)
                nc.sync.dma_start(lab_sb[:], lab_ext[t][:])

                h_sb = hp.tile([128, K * TN], bf16)
                s_ps = sps.tile([128, 64], f32, name="s_ps", tag="sps")

                def emit_xcorr(c):
                    # 0.2(Wv)^T x correction for a relu-only (DVE) chunk;
                    # depends only on x, so it's always-ready PE filler.
                    # First writer of each column -> start=True.
                    for k in range(4 * c, 4 * c + 4):
                        for s in range(NSUB):
                            col = 32 * s + k
                            base = k * TN + s * 128
                            nc.tensor.matmul(
                                s_ps[:, col:col + 1],
                                x_sb[:, base:base + 128], wv02[:],
                                start=True, stop=False,
                            )

                def emit_scores(c):
                    # h-term for chunk c's k values (ready once relu c done)
                    dve = SCHED[c] == "D"
                    for k in range(4 * c, 4 * c + 4):
                        for s in range(NSUB):
                            col = 32 * s + k
                            base = k * TN + s * 128
                            nc.tensor.matmul(
                                s_ps[:, col:col + 1],
                                h_sb[:, base:base + 128],
                                v08[:] if dve else v10[:],
                                start=not dve, stop=True,
                            )

                for c in range(NCH):
                    z_ps = zps.tile([128, 1024], f32, name="z_ps", tag="z")
                    nc.tensor.matmul(
                        z_ps[:, 0:512], W_sb[:], x_sb[:, c * 1024:c * 1024 + 512]
                    )
                    nc.tensor.matmul(
                        z_ps[:, 512:1024], W_sb[:],
                        x_sb[:, c * 1024 + 512:(c + 1) * 1024],
                    )
                    if SCHED[c] == "D":
                        emit_xcorr(c)
                    hc = h_sb[:, c * 1024:(c + 1) * 1024]
                    if SCHED[c] == "A":
                        nc.scalar.activation(hc, z_ps[:], AF.Prelu, alpha=NEG)
                    else:
                        nc.vector.tensor_scalar_max(hc, z_ps[:], 0.0)
                    if c == 0 and prev is not None:
                        emit_head(prev)
                    if c == 1 and prev is not None:
                        emit_transp(prev)
                    if c in (2, 3, 4, 5) and prev is not None:
                        emit_agg(prev, c - 2)
                    if c >= 1:
                        emit_scores(c - 1)
                    if c == 6 and prev is not None:
                        emit_out(prev)
                emit_scores(NCH - 1)

                prev = {"t": t, "s_ps": s_ps, "lab_sb": lab_sb}

            # drain the last tile
            emit_head(prev)
            emit_transp(prev)
            for q in range(4):
                emit_agg(prev, q)
            emit_out(prev)
    nc.compile()
    return nc


def shard_x(x, nt=NT, nper=NPER):
    import ml_dtypes

    f8 = ml_dtypes.float8_e3m4
    xs = np.zeros((nt * TN, K, D), f8)
    xs[:nper] = x.astype(f8)
    # [t, n, k, d] -> [t, d, k, n] -> col = k*TN + n
    return np.ascontiguousarray(
        xs.reshape(nt, TN, K, D).transpose(0, 3, 2, 1)
    ).reshape(nt, 128, K * TN)


def shard_lab(lab, nt=NT, nper=NPER):
    import ml_dtypes

    f8 = ml_dtypes.float8_e3m4
    ls = np.zeros((nt * TN, K, L), f8)
    ls[:nper] = lab.astype(f8)
    # group g = node//4, j = node%4: row = 32j + k, col = 64g + l
    l5 = ls.reshape(nt, 64, 4, K, L)          # [t, g, j, k, l]
    return np.ascontiguousarray(
        l5.transpose(0, 2, 3, 1, 4)           # [t, j, k, g, l]
    ).reshape(nt, 128, 64 * L)


def make_in_maps(inputs):
    import ml_dtypes

    bf16 = ml_dtypes.bfloat16
    x = np.asarray(inputs["para_neighbors"], np.float32)
    lab = np.asarray(inputs["para_nei_labels"], np.float32)
    Wm = np.ascontiguousarray(np.asarray(inputs["linear"], np.float32))
    v = np.ascontiguousarray(np.asarray(inputs["e_vec"], np.float32))
    b = np.asarray(inputs["bias"], np.float32).reshape(K)

    Wb = Wm.astype(bf16).astype(np.float32)
    vb = v.astype(bf16).astype(np.float32)
    W16 = np.ascontiguousarray(Wm.astype(bf16))
    v10 = np.ascontiguousarray(vb.astype(bf16))
    v08 = np.ascontiguousarray((0.8 * vb).astype(bf16))
    # 0.2*(W@v) from the bf16-rounded W/v so the correction matches the PE's z
    wv02 = np.ascontiguousarray((NEG * (Wb @ vb)).astype(bf16))
    # b64[p, 32s+k] = bias[k] (same for every partition row)
    b64 = np.ascontiguousarray(
        np.tile(np.concatenate([b, b])[None, :], (128, 1))
    ).astype(np.float32)

    in_maps = []
    for i in range(NCORES):
        xf = shard_x(x[i * NPER:(i + 1) * NPER])
        lf = shard_lab(lab[i * NPER:(i + 1) * NPER])
        in_maps.append({
            "x": xf, "lab": lf, "w": W16, "v10": v10, "v08": v08,
            "wv02": wv02, "b64": b64,
        })
    return in_maps


def unshard_output(res_i):
    # out[t, l, c] = raw_sum for node n = t*TN + c; sums[t, 0, c] likewise.
    # Softmax normalization happens here.
    o = np.asarray(res_i["out"]).astype(np.float32)      # [nt, L, TN]
    sums = np.asarray(res_i["sums"]).astype(np.float32)  # [nt, 1, TN]
    raw = o.transpose(0, 2, 1).reshape(NT * TN, L)
    s = sums.reshape(NT * TN)
    return (raw[:NPER] / s[:NPER, None]).astype(np.float32)


def kernel(para_neighbors, para_nei_labels, linear, e_vec, bias):
    from concourse.bass_utils import run_bass_kernel_spmd

    global LAST_RESULT
    with_bias = bool(np.any(np.asarray(bias)))
    key = ("nc", with_bias)
    if key not in _cache:
        _cache[key] = build(with_bias)
        _cache["nc"] = _cache[key]
    nc = _cache[key]

    in_maps = make_in_maps({
        "para_neighbors": para_neighbors, "para_nei_labels": para_nei_labels,
        "linear": linear, "e_vec": e_vec, "bias": bias,
    })
    res = run_bass_kernel_spmd(nc, in_maps, core_ids=list(range(NCORES)))
    LAST_RESULT = res
    outs = [unshard_output(res.results[i]) for i in range(NCORES)]
    return np.ascontiguousarray(np.concatenate(outs, axis=0))


# revision 8
# speedup vs baseline: 1.8468x; 1.3133x over previous
"""Trainium2 Bass kernel: GAT-style attention layer, data-parallel over 8 NeuronCores.

Reference computation (per node n, K=32 neighbors, D=128 features, L=64 labels):
    h     = lrelu(x @ W)                  [N,K,D]
    e     = lrelu(h @ v + bias)           [N,K,1]
    alpha = softmax_k(e)                  [N,K]
    out   = sum_k alpha[n,k] * labels[n,k,:]   [N,L]

Sharding: pure data parallel over nodes (6250/core, zero-padded to 6400).

Structure (v3): every contraction over a 128-partition axis rides the PE with
the LARGE tensor as the *stationary* operand and a 1..4-column moving operand
(PE engine time ~ moving columns only):
  mm1    z^T[e,(k,n)] = W^T @ x^T      16x512-col fp8 matmuls / 256-node tile
  act    h = lrelu(z) (ScalarE chunks, full Prelu) or relu(z) (VectorE
         chunks; the 0.2z part is restored by an x-correction matmul)
  score  s[n, 32s+k]: per (k,sub) a 1-column matmul with the h slice as
         stationary (+ 0.2(Wv)^T x correction for VectorE chunks)
  e/exp  ACT Prelu + Exp on [128,64]
  alphaT 8 32x32 PE transposes place node-quarter j's exp-weights at
         partition block 32j of a persistent (startup-zeroed) PSUM tile;
         one DVE copy -> aT4 [128, 256n] with zeros off-block
  agg    out^T[l, 4 nodes] per matmul: stationary = 4 nodes' label blocks
         stacked [128=(4x32k), 64l] fp8; off-block zeros in aT4 kill the
         cross-node terms. 64 Ldweights+matmuls per tile.
  sums   gpsimd partition-reduce of aT4 -> [1, 256]; softmax 1/sum applied
         on the HOST (sums are DMA'd out, 1KB/tile)

Quantization: x fp8-e3m4, labels fp8-e3m4, weights bf16, out bf16.
Per-core DRAM traffic ~39MB; DMA floor ~4.5us per 256-node tile.
"""
import sys

sys.path.insert(0, "/opt/trn_rl_repo")
import numpy as np

N, K, D, L = 50000, 32, 128, 64
NEG = 0.2
NCORES = 8
NPER = N // NCORES          # 6250
TN = 256                    # nodes per tile
NSUB = TN // 128            # sub-tiles of 128 nodes
NPAD = 6400                 # padded nodes per core
NT = NPAD // TN             # 25 tiles
NCH = 8                     # mm1 chunks per tile (1024 cols each)

# relu-chunk engine schedule: 'A' = ScalarE (full Prelu), 'D' = VectorE
# (relu-only + PE x-correction). Chunk 0 must be 'D' so ACT starts each tile
# with the previous tile's Prelu/Exp.
SCHED = "DADADADA"

LAST_RESULT = None
_cache = {}


def build(with_bias):
    import concourse.bass as bass
    import concourse.tile as tile
    from concourse import bacc, mybir

    f32 = mybir.dt.float32
    bf16 = mybir.dt.bfloat16
    f8 = mybir.dt.float8e3
    AF = mybir.ActivationFunctionType
    OP = mybir.AluOpType
    PSUM = bass.MemorySpace.PSUM
    nt = NT

    nc = bacc.Bacc(
        "TRN2", target_bir_lowering=False, debug=False, num_devices=NCORES
    )
    x_ext = nc.declare_dram_parameter("x", [nt, 128, K * TN], f8, False)
    lab_ext = nc.declare_dram_parameter("lab", [nt, 128, 64 * 65], f8, False)
    w_ext = nc.declare_dram_parameter("w", [D, D], bf16, False)
    v10_ext = nc.declare_dram_parameter("v10", [D, 1], bf16, False)
    v08_ext = nc.declare_dram_parameter("v08", [D, 1], bf16, False)
    wv02_ext = nc.declare_dram_parameter("wv02", [D, 1], bf16, False)
    b64_ext = nc.declare_dram_parameter("b64", [128, 64], f32, False)
    out_ext = nc.declare_dram_parameter("out", [nt, L + 1, TN], bf16, isOutput=True)

    with tile.TileContext(nc) as tc:
        with (
            tc.tile_pool(name="const", bufs=1) as const,
            tc.tile_pool(name="xp", bufs=3) as xp,
            tc.tile_pool(name="labp", bufs=2) as labp,
            tc.tile_pool(name="hp", bufs=2) as hp,
            tc.tile_pool(name="smallp", bufs=2) as smallp,
            tc.tile_pool(name="outp", bufs=2) as outp,
            tc.tile_pool(name="zps", bufs=3, space=PSUM) as zps,
            tc.tile_pool(name="smps", bufs=1, space=PSUM) as smps,
        ):
            W_sb = const.tile([128, 128], bf16)
            nc.sync.dma_start(W_sb[:], w_ext[:])
            v10 = const.tile([128, 1], bf16)
            nc.sync.dma_start(v10[:], v10_ext[:])
            v08 = const.tile([128, 1], bf16)
            nc.sync.dma_start(v08[:], v08_ext[:])
            wv02 = const.tile([128, 1], bf16)
            nc.sync.dma_start(wv02[:], wv02_ext[:])
            if with_bias:
                b64 = const.tile([128, 64], f32)
                nc.sync.dma_start(b64[:], b64_ext[:])
            ones = const.tile([128, 128], bf16)
            nc.vector.memset(ones[:], 1.0)
            ident = const.tile([128, 128], bf16)         # identity matrix
            nc.gpsimd.affine_select(
                ident[:], ones[:], pattern=[[1, 128]],
                compare_op=OP.is_equal, fill=0.0, base=0, channel_multiplier=-1,
            )
            # One PSUM bank holds all the small tiles, manually carved:
            # cols 0:64 / 64:128 = s_ps (alternating per tile), 128:384 =
            # o_ps [65,256], 384:512 bitcast bf16 = aT4_ps [128,256].
            smalls = smps.tile([128, 512], f32, name="smalls", tag="smalls")
            s_ps_ab = (smalls[:, 0:64], smalls[:, 64:128])
            o_ps_ap = smalls[0:65, 128:384]
            aT4_ps = smalls[:, 384:512].bitcast(bf16)
            # persistent exp-weight transpose target: node-quarter j occupies
            # partition block 32j; everything off-block is zeroed ONCE here
            # and never written again, so cross-node terms in the batched agg
            # matmul multiply against exact zeros.
            nc.vector.memset(aT4_ps, 0.0)

            # PE warmup burst: dummy matmuls (deps ready ~1us in) while the
            # first x tile loads; the p-state model needs ~3.4us of early PE
            # activity to reach the full 2.4GHz clock.
            warm_ps = zps.tile([128, 1024], f32, name="warm_ps", tag="z")
            for _ in range(32):
                nc.tensor.matmul(
                    warm_ps[:, 0:128], W_sb[:], ones[:], skip_group_check=True
                )

            prev = None   # state of tile t-1 awaiting its score/agg tail

            def emit_head(st):
                """e = lrelu(s) (+bias), w = exp(e). First ACT ops of a tile."""
                s_ps = st["s_ps"]
                e_sb = smallp.tile([128, 64], f32, name="e_sb", tag="e_sb")
                if with_bias:
                    sb = smallp.tile([128, 64], f32, name="sb", tag="sb")
                    nc.vector.tensor_tensor(sb[:], s_ps, b64[:], OP.add)
                    nc.scalar.activation(e_sb[:], sb[:], AF.Prelu, alpha=NEG)
                else:
                    nc.scalar.activation(e_sb[:], s_ps, AF.Prelu, alpha=NEG)
                w_sb = smallp.tile([128, 64], bf16, name="w_sb", tag="w_sb")
                nc.scalar.activation(w_sb[:], e_sb[:], AF.Exp)
                st["w_sb"] = w_sb

            def emit_transp(st):
                """8 32x32 transposes: sub s node-quarter j -> aT4_ps rows
                [32j,32j+32), cols [128s+32j, +32). Then one DVE copy to SBUF
                (zeros off-block come along) + gpsimd per-node sums + DMA."""
                w_sb = st["w_sb"]
                for s in range(NSUB):
                    for j in range(4):
                        nc.tensor.transpose(
                            aT4_ps[32 * j:32 * j + 32,
                                    128 * s + 32 * j:128 * s + 32 * j + 32],
                            w_sb[32 * j:32 * j + 32, 32 * s:32 * s + 32],
                            ident[32 * j:32 * j + 32, 32 * j:32 * j + 32],
                            tile_position=(32 * j, 32 * j),
                        )
                aT4 = smallp.tile([128, TN], bf16, name="aT4", tag="aT4")
                nc.vector.tensor_copy(aT4[:], aT4_ps)
                st["aT4"] = aT4
                st["o_ps"] = o_ps_ap

            def emit_agg(st, q):
                """Aggregation for node groups [16q, 16q+16): one 4-column
                matmul per group of 4 nodes; stationary = their label blocks
                stacked [128, 64] fp8 + a 65th all-ones column whose output
                row is the per-node exp-weight sum (softmax denominator)."""
                lab_sb, aT4, o_ps = st["lab_sb"], st["aT4"], st["o_ps"]
                for g in range(16 * q, 16 * q + 16):
                    nc.tensor.matmul(
                        o_ps[:, 4 * g:4 * g + 4],
                        lab_sb[:, 65 * g:65 * g + 65],
                        aT4[:, 4 * g:4 * g + 4],
                    )

            def emit_out(st):
                o_sb = outp.tile([L + 1, TN], bf16, name="o_sb", tag="o_sb")
                nc.scalar.activation(o_sb[:], st["o_ps"], AF.Copy)
                nc.sync.dma_start(out_ext[st["t"]], o_sb[:])

            for t in range(nt):
                x_sb = xp.tile([128, K * TN], f8)
                if t == 0:
                    # quarter the first x load so chunk 0's matmuls start
                    # after ~256KB instead of a full 1MB
                    qn = K * TN // 4
                    for qi in range(4):
                        nc.sync.dma_start(
                            x_sb[:, qi * qn:(qi + 1) * qn],
                            x_ext[t][:, qi * qn:(qi + 1) * qn],
                        )
                else:
                    nc.sync.dma_start(x_sb[:], x_ext[t][:])
                lab_sb = labp.tile([128, 64 * 65], f8)
                nc.sync.dma_start(lab_sb[:], lab_ext[t][:])

                h_sb = hp.tile([128, K * TN], bf16)
                s_ps = s_ps_ab[t % 2]

                def emit_xcorr(c):
                    # 0.2(Wv)^T x correction for a relu-only (DVE) chunk;
                    # depends only on x, so it's always-ready PE filler.
                    # First writer of each column -> start=True.
                    for k in range(4 * c, 4 * c + 4):
                        for s in range(NSUB):
                            col = 32 * s + k
                            base = k * TN + s * 128
                            nc.tensor.matmul(
                                s_ps[:, col:col + 1],
                                x_sb[:, base:base + 128], wv02[:],
                                start=True, stop=False,
                            )

                def emit_scores(c):
                    # h-term for chunk c's k values (ready once relu c done)
                    dve = SCHED[c] == "D"
                    for k in range(4 * c, 4 * c + 4):
                        for s in range(NSUB):
                            col = 32 * s + k
                            base = k * TN + s * 128
                            nc.tensor.matmul(
                                s_ps[:, col:col + 1],
                                h_sb[:, base:base + 128],
                                v08[:] if dve else v10[:],
                                start=not dve, stop=True,
                            )

                for c in range(NCH):
                    z_ps = zps.tile([128, 1024], f32, name="z_ps", tag="z")
                    nc.tensor.matmul(
                        z_ps[:, 0:512], W_sb[:], x_sb[:, c * 1024:c * 1024 + 512]
                    )
                    nc.tensor.matmul(
                        z_ps[:, 512:1024], W_sb[:],
                        x_sb[:, c * 1024 + 512:(c + 1) * 1024],
                    )
                    if SCHED[c] == "D":
                        emit_xcorr(c)
                    hc = h_sb[:, c * 1024:(c + 1) * 1024]
                    if SCHED[c] == "A":
                        nc.scalar.activation(hc, z_ps[:], AF.Prelu, alpha=NEG)
                    else:
                        nc.vector.tensor_scalar_max(hc, z_ps[:], 0.0)
                    if c == 0 and prev is not None:
                        emit_head(prev)
                    if c == 1 and prev is not None:
                        emit_transp(prev)
                    if c in (2, 3, 4, 5) and prev is not None:
                        emit_agg(prev, c - 2)
                    if c >= 2:
                        emit_scores(c - 2)
                    if c == 6 and prev is not None:
                        emit_out(prev)
                emit_scores(NCH - 2)
                emit_scores(NCH - 1)

                prev = {"t": t, "s_ps": s_ps, "lab_sb": lab_sb}

            # drain the last tile
            emit_head(prev)
            emit_transp(prev)
            for q in range(4):
                emit_agg(prev, q)
            emit_out(prev)
    nc.compile()
    return nc


def shard_x(x, nt=NT, nper=NPER):
    import ml_dtypes

    f8 = ml_dtypes.float8_e3m4
    xs = np.zeros((nt * TN, K, D), f8)
    xs[:nper] = x.astype(f8)
    # [t, n, k, d] -> [t, d, k, n] -> col = k*TN + n
    return np.ascontiguousarray(
        xs.reshape(nt, TN, K, D).transpose(0, 3, 2, 1)
    ).reshape(nt, 128, K * TN)


def shard_lab(lab, nt=NT, nper=NPER):
    import ml_dtypes

    f8 = ml_dtypes.float8_e3m4
    ls = np.zeros((nt * TN, K, L + 1), f8)
    ls[:nper, :, :L] = lab.astype(f8)
    ls[:, :, L] = f8(1.0)   # ones column -> per-node exp-weight sums
    # group g = node//4, j = node%4: row = 32j + k, col = 65g + l
    l5 = ls.reshape(nt, 64, 4, K, L + 1)      # [t, g, j, k, l]
    return np.ascontiguousarray(
        l5.transpose(0, 2, 3, 1, 4)           # [t, j, k, g, l]
    ).reshape(nt, 128, 64 * 65)


def make_in_maps(inputs):
    import ml_dtypes

    bf16 = ml_dtypes.bfloat16
    x = np.asarray(inputs["para_neighbors"], np.float32)
    lab = np.asarray(inputs["para_nei_labels"], np.float32)
    Wm = np.ascontiguousarray(np.asarray(inputs["linear"], np.float32))
    v = np.ascontiguousarray(np.asarray(inputs["e_vec"], np.float32))
    b = np.asarray(inputs["bias"], np.float32).reshape(K)

    Wb = Wm.astype(bf16).astype(np.float32)
    vb = v.astype(bf16).astype(np.float32)
    W16 = np.ascontiguousarray(Wm.astype(bf16))
    v10 = np.ascontiguousarray(vb.astype(bf16))
    v08 = np.ascontiguousarray((0.8 * vb).astype(bf16))
    # 0.2*(W@v) from the bf16-rounded W/v so the correction matches the PE's z
    wv02 = np.ascontiguousarray((NEG * (Wb @ vb)).astype(bf16))
    # b64[p, 32s+k] = bias[k] (same for every partition row)
    b64 = np.ascontiguousarray(
        np.tile(np.concatenate([b, b])[None, :], (128, 1))
    ).astype(np.float32)

    in_maps = []
    for i in range(NCORES):
        xf = shard_x(x[i * NPER:(i + 1) * NPER])
        lf = shard_lab(lab[i * NPER:(i + 1) * NPER])
        in_maps.append({
            "x": xf, "lab": lf, "w": W16, "v10": v10, "v08": v08,
            "wv02": wv02, "b64": b64,
        })
    return in_maps


def unshard_output(res_i):
    # out[t, l, c] = raw_sum for node n = t*TN + c; row L = exp-weight sum.
    # Softmax normalization happens here.
    o = np.asarray(res_i["out"]).astype(np.float32)      # [nt, L+1, TN]
    raw = o[:, :L].transpose(0, 2, 1).reshape(NT * TN, L)
    s = o[:, L].reshape(NT * TN)
    return (raw[:NPER] / s[:NPER, None]).astype(np.float32)


def kernel(para_neighbors, para_nei_labels, linear, e_vec, bias):
    from concourse.bass_utils import run_bass_kernel_spmd

    global LAST_RESULT
    with_bias = bool(np.any(np.asarray(bias)))
    key = ("nc", with_bias)
    if key not in _cache:
        _cache[key] = build(with_bias)
        _cache["nc"] = _cache[key]
    nc = _cache[key]

    in_maps = make_in_maps({
        "para_neighbors": para_neighbors, "para_nei_labels": para_nei_labels,
        "linear": linear, "e_vec": e_vec, "bias": bias,
    })
    res = run_bass_kernel_spmd(nc, in_maps, core_ids=list(range(NCORES)))
    LAST_RESULT = res
    outs = [unshard_output(res.results[i]) for i in range(NCORES)]
    return np.ascontiguousarray(np.concatenate(outs, axis=0))


# revision 9
# speedup vs baseline: 1.8671x; 1.0110x over previous
"""Trainium2 Bass kernel: GAT-style attention layer, data-parallel over 8 NeuronCores.

Reference computation (per node n, K=32 neighbors, D=128 features, L=64 labels):
    h     = lrelu(x @ W)                  [N,K,D]
    e     = lrelu(h @ v + bias)           [N,K,1]
    alpha = softmax_k(e)                  [N,K]
    out   = sum_k alpha[n,k] * labels[n,k,:]   [N,L]

Sharding: pure data parallel over nodes (6250/core, zero-padded to 6400).

Structure (v3): every contraction over a 128-partition axis rides the PE with
the LARGE tensor as the *stationary* operand and a 1..4-column moving operand
(PE engine time ~ moving columns only):
  mm1    z^T[e,(k,n)] = W^T @ x^T      16x512-col fp8 matmuls / 256-node tile
  act    h = lrelu(z) (ScalarE chunks, full Prelu) or relu(z) (VectorE
         chunks; the 0.2z part is restored by an x-correction matmul)
  score  s[n, 32s+k]: per (k,sub) a 1-column matmul with the h slice as
         stationary (+ 0.2(Wv)^T x correction for VectorE chunks)
  e/exp  ACT Prelu + Exp on [128,64]
  alphaT 8 32x32 PE transposes place node-quarter j's exp-weights at
         partition block 32j of a persistent (startup-zeroed) PSUM tile;
         one DVE copy -> aT4 [128, 256n] with zeros off-block
  agg    out^T[l, 4 nodes] per matmul: stationary = 4 nodes' label blocks
         stacked [128=(4x32k), 64l] fp8; off-block zeros in aT4 kill the
         cross-node terms. 64 Ldweights+matmuls per tile.
  sums   gpsimd partition-reduce of aT4 -> [1, 256]; softmax 1/sum applied
         on the HOST (sums are DMA'd out, 1KB/tile)

Quantization: x fp8-e3m4, labels fp8-e3m4, weights bf16, out bf16.
Per-core DRAM traffic ~39MB; DMA floor ~4.5us per 256-node tile.
"""
import sys

sys.path.insert(0, "/opt/trn_rl_repo")
import numpy as np

N, K, D, L = 50000, 32, 128, 64
NEG = 0.2
NCORES = 8
NPER = N // NCORES          # 6250
TN = 256                    # nodes per tile
NSUB = TN // 128            # sub-tiles of 128 nodes
NPAD = 6400                 # padded nodes per core
NT = NPAD // TN             # 25 tiles
NCH = 8                     # mm1 chunks per tile (1024 cols each)

# relu-chunk engine schedule: 'A' = ScalarE (full Prelu), 'D' = VectorE
# (relu-only + PE x-correction). Chunk 0 must be 'D' so ACT starts each tile
# with the previous tile's Prelu/Exp.
SCHED = "DADADADA"

LAST_RESULT = None
_cache = {}


def build(with_bias):
    import concourse.bass as bass
    import concourse.tile as tile
    from concourse import bacc, mybir

    f32 = mybir.dt.float32
    bf16 = mybir.dt.bfloat16
    f8 = mybir.dt.float8e3
    AF = mybir.ActivationFunctionType
    OP = mybir.AluOpType
    PSUM = bass.MemorySpace.PSUM
    nt = NT

    nc = bacc.Bacc(
        "TRN2", target_bir_lowering=False, debug=False, num_devices=NCORES
    )
    x_ext = nc.declare_dram_parameter("x", [nt, 128, K * TN], f8, False)
    lab_ext = nc.declare_dram_parameter("lab", [nt, 128, 64 * 65], f8, False)
    w_ext = nc.declare_dram_parameter("w", [D, D], bf16, False)
    v10_ext = nc.declare_dram_parameter("v10", [D, 1], bf16, False)
    v08_ext = nc.declare_dram_parameter("v08", [D, 1], bf16, False)
    wv02_ext = nc.declare_dram_parameter("wv02", [D, 1], bf16, False)
    b64_ext = nc.declare_dram_parameter("b64", [128, 64], f32, False)
    out_ext = nc.declare_dram_parameter("out", [nt, L + 1, TN], bf16, isOutput=True)

    with tile.TileContext(nc) as tc:
        with (
            tc.tile_pool(name="const", bufs=1) as const,
            tc.tile_pool(name="xp", bufs=3) as xp,
            tc.tile_pool(name="labp", bufs=2) as labp,
            tc.tile_pool(name="hp", bufs=2) as hp,
            tc.tile_pool(name="smallp", bufs=2) as smallp,
            tc.tile_pool(name="outp", bufs=2) as outp,
            tc.tile_pool(name="zps", bufs=3, space=PSUM) as zps,
            tc.tile_pool(name="smps", bufs=1, space=PSUM) as smps,
        ):
            W_sb = const.tile([128, 128], bf16)
            nc.sync.dma_start(W_sb[:], w_ext[:])
            v10 = const.tile([128, 1], bf16)
            nc.sync.dma_start(v10[:], v10_ext[:])
            v08 = const.tile([128, 1], bf16)
            nc.sync.dma_start(v08[:], v08_ext[:])
            wv02 = const.tile([128, 1], bf16)
            nc.sync.dma_start(wv02[:], wv02_ext[:])
            if with_bias:
                b64 = const.tile([128, 64], f32)
                nc.sync.dma_start(b64[:], b64_ext[:])
            ones = const.tile([128, 128], bf16)
            nc.vector.memset(ones[:], 1.0)
            ident = const.tile([128, 128], bf16)         # identity matrix
            nc.gpsimd.affine_select(
                ident[:], ones[:], pattern=[[1, 128]],
                compare_op=OP.is_equal, fill=0.0, base=0, channel_multiplier=-1,
            )
            # One PSUM bank holds all the small tiles, manually carved:
            # cols 0:64 / 64:128 = s_ps (alternating per tile), 128:384 =
            # o_ps [65,256], 384:512 bitcast bf16 = aT4_ps [128,256].
            smalls = smps.tile([128, 512], f32, name="smalls", tag="smalls")
            s_ps_ab = (smalls[:, 0:64], smalls[:, 64:128])
            o_ps_ap = smalls[0:65, 128:384]
            aT4_ps = smalls[:, 384:512].bitcast(bf16)
            # persistent exp-weight transpose target: node-quarter j occupies
            # partition block 32j; everything off-block is zeroed ONCE here
            # and never written again, so cross-node terms in the batched agg
            # matmul multiply against exact zeros.
            nc.vector.memset(aT4_ps, 0.0)

            # PE warmup burst: dummy matmuls (deps ready ~1us in) while the
            # first x tile loads; the p-state model needs ~3.4us of early PE
            # activity to reach the full 2.4GHz clock.
            warm_ps = zps.tile([128, 1024], f32, name="warm_ps", tag="z")
            for _ in range(32):
                nc.tensor.matmul(
                    warm_ps[:, 0:128], W_sb[:], ones[:], skip_group_check=True
                )

            prev = None   # state of tile t-1 awaiting its score/agg tail

            def emit_head(st):
                """e = lrelu(s) (+bias), w = exp(e). First ACT ops of a tile."""
                ns = st["subs"]
                s_ps = st["s_ps"][:, 0:32 * ns]
                e_sb = smallp.tile([128, 64], f32, name="e_sb", tag="e_sb")
                if with_bias:
                    sb = smallp.tile([128, 64], f32, name="sb", tag="sb")
                    nc.vector.tensor_tensor(sb[:, 0:32 * ns], s_ps, b64[:, 0:32 * ns], OP.add)
                    nc.scalar.activation(e_sb[:, 0:32 * ns], sb[:, 0:32 * ns], AF.Prelu, alpha=NEG)
                else:
                    nc.scalar.activation(e_sb[:, 0:32 * ns], s_ps, AF.Prelu, alpha=NEG)
                w_sb = smallp.tile([128, 64], bf16, name="w_sb", tag="w_sb")
                nc.scalar.activation(w_sb[:, 0:32 * ns], e_sb[:, 0:32 * ns], AF.Exp)
                st["w_sb"] = w_sb

            def emit_transp(st):
                """8 32x32 transposes: sub s node-quarter j -> aT4_ps rows
                [32j,32j+32), cols [128s+32j, +32). Then one DVE copy to SBUF
                (zeros off-block come along) + gpsimd per-node sums + DMA."""
                w_sb = st["w_sb"]
                for s in range(st["subs"]):
                    for j in range(4):
                        nc.tensor.transpose(
                            aT4_ps[32 * j:32 * j + 32,
                                    128 * s + 32 * j:128 * s + 32 * j + 32],
                            w_sb[32 * j:32 * j + 32, 32 * s:32 * s + 32],
                            ident[32 * j:32 * j + 32, 32 * j:32 * j + 32],
                            tile_position=(32 * j, 32 * j),
                        )
                aT4 = smallp.tile([128, TN], bf16, name="aT4", tag="aT4")
                nc.vector.tensor_copy(
                    aT4[:, 0:128 * st["subs"]], aT4_ps[:, 0:128 * st["subs"]]
                )
                st["aT4"] = aT4
                st["o_ps"] = o_ps_ap

            def emit_agg(st, q):
                """Aggregation for node groups [16q, 16q+16): one 4-column
                matmul per group of 4 nodes; stationary = their label blocks
                stacked [128, 64] fp8 + a 65th all-ones column whose output
                row is the per-node exp-weight sum (softmax denominator)."""
                lab_sb, aT4, o_ps = st["lab_sb"], st["aT4"], st["o_ps"]
                gmax = 32 * st["subs"]
                for g in range(16 * q, min(16 * q + 16, gmax)):
                    nc.tensor.matmul(
                        o_ps[:, 4 * g:4 * g + 4],
                        lab_sb[:, 65 * g:65 * g + 65],
                        aT4[:, 4 * g:4 * g + 4],
                    )

            def emit_out(st):
                w = 128 * st["subs"]
                o_sb = outp.tile([L + 1, TN], bf16, name="o_sb", tag="o_sb")
                nc.scalar.activation(o_sb[:, 0:w], st["o_ps"][:, 0:w], AF.Copy)
                nc.sync.dma_start(out_ext[st["t"]][:, 0:w], o_sb[:, 0:w])

            for t in range(nt):
                x_sb = xp.tile([128, K * TN], f8)
                if t == 0:
                    # quarter the first x load so chunk 0's matmuls start
                    # after ~256KB instead of a full 1MB
                    qn = K * TN // 4
                    for qi in range(4):
                        nc.sync.dma_start(
                            x_sb[:, qi * qn:(qi + 1) * qn],
                            x_ext[t][:, qi * qn:(qi + 1) * qn],
                        )
                else:
                    nc.sync.dma_start(x_sb[:], x_ext[t][:])
                lab_sb = labp.tile([128, 64 * 65], f8)
                nc.sync.dma_start(lab_sb[:], lab_ext[t][:])

                h_sb = hp.tile([128, K * TN], bf16)
                s_ps = s_ps_ab[t % 2]
                tsubs = 1 if t == nt - 1 else NSUB

                def emit_xcorr(c):
                    # 0.2(Wv)^T x correction for a relu-only (DVE) chunk;
                    # depends only on x, so it's always-ready PE filler.
                    # First writer of each column -> start=True.
                    for k in range(4 * c, 4 * c + 4):
                        for s in range(tsubs):
                            col = 32 * s + k
                            base = k * TN + s * 128
                            nc.tensor.matmul(
                                s_ps[:, col:col + 1],
                                x_sb[:, base:base + 128], wv02[:],
                                start=True, stop=False,
                            )

                def emit_scores(c):
                    # h-term for chunk c's k values (ready once relu c done)
                    dve = SCHED[c] == "D"
                    for k in range(4 * c, 4 * c + 4):
                        for s in range(tsubs):
                            col = 32 * s + k
                            base = k * TN + s * 128
                            nc.tensor.matmul(
                                s_ps[:, col:col + 1],
                                h_sb[:, base:base + 128],
                                v08[:] if dve else v10[:],
                                start=not dve, stop=True,
                            )

                for c in range(NCH):
                    z_ps = zps.tile([128, 1024], f32, name="z_ps", tag="z")
                    nc.tensor.matmul(
                        z_ps[:, 0:512], W_sb[:], x_sb[:, c * 1024:c * 1024 + 512]
                    )
                    nc.tensor.matmul(
                        z_ps[:, 512:1024], W_sb[:],
                        x_sb[:, c * 1024 + 512:(c + 1) * 1024],
                    )
                    if SCHED[c] == "D":
                        emit_xcorr(c)
                    if t == nt - 1:
                        # padding-only sub-tiles: activate the first 128
                        # nodes per k only (the rest is never read)
                        zv = z_ps[:].rearrange("p (k n) -> p k n", k=4)[:, :, 0:128]
                        hv = h_sb[:, c * 1024:(c + 1) * 1024].rearrange(
                            "p (k n) -> p k n", k=4)[:, :, 0:128]
                    else:
                        zv = z_ps[:]
                        hv = h_sb[:, c * 1024:(c + 1) * 1024]
                    if SCHED[c] == "A":
                        nc.scalar.activation(hv, zv, AF.Prelu, alpha=NEG)
                    else:
                        nc.vector.tensor_scalar_max(hv, zv, 0.0)
                    if c == 0 and prev is not None:
                        emit_head(prev)
                    if c == 1 and prev is not None:
                        emit_transp(prev)
                    if c in (2, 3, 4, 5) and prev is not None:
                        emit_agg(prev, c - 2)
                    if c >= 2:
                        emit_scores(c - 2)
                    if c == 6 and prev is not None:
                        emit_out(prev)
                emit_scores(NCH - 2)
                emit_scores(NCH - 1)

                prev = {"t": t, "s_ps": s_ps, "lab_sb": lab_sb, "subs": tsubs}

            # drain the last tile
            emit_head(prev)
            emit_transp(prev)
            for q in range(4):
                emit_agg(prev, q)
            emit_out(prev)
    nc.compile()
    return nc


def shard_x(x, nt=NT, nper=NPER):
    import ml_dtypes

    f8 = ml_dtypes.float8_e3m4
    xs = np.zeros((nt * TN, K, D), f8)
    xs[:nper] = x.astype(f8)
    # [t, n, k, d] -> [t, d, k, n] -> col = k*TN + n
    return np.ascontiguousarray(
        xs.reshape(nt, TN, K, D).transpose(0, 3, 2, 1)
    ).reshape(nt, 128, K * TN)


def shard_lab(lab, nt=NT, nper=NPER):
    import ml_dtypes

    f8 = ml_dtypes.float8_e3m4
    ls = np.zeros((nt * TN, K, L + 1), f8)
    ls[:nper, :, :L] = lab.astype(f8)
    ls[:, :, L] = f8(1.0)   # ones column -> per-node exp-weight sums
    # group g = node//4, j = node%4: row = 32j + k, col = 65g + l
    l5 = ls.reshape(nt, 64, 4, K, L + 1)      # [t, g, j, k, l]
    return np.ascontiguousarray(
        l5.transpose(0, 2, 3, 1, 4)           # [t, j, k, g, l]
    ).reshape(nt, 128, 64 * 65)


def make_in_maps(inputs):
    import ml_dtypes

    bf16 = ml_dtypes.bfloat16
    x = np.asarray(inputs["para_neighbors"], np.float32)
    lab = np.asarray(inputs["para_nei_labels"], np.float32)
    Wm = np.ascontiguousarray(np.asarray(inputs["linear"], np.float32))
    v = np.ascontiguousarray(np.asarray(inputs["e_vec"], np.float32))
    b = np.asarray(inputs["bias"], np.float32).reshape(K)

    Wb = Wm.astype(bf16).astype(np.float32)
    vb = v.astype(bf16).astype(np.float32)
    W16 = np.ascontiguousarray(Wm.astype(bf16))
    v10 = np.ascontiguousarray(vb.astype(bf16))
    v08 = np.ascontiguousarray((0.8 * vb).astype(bf16))
    # 0.2*(W@v) from the bf16-rounded W/v so the correction matches the PE's z
    wv02 = np.ascontiguousarray((NEG * (Wb @ vb)).astype(bf16))
    # b64[p, 32s+k] = bias[k] (same for every partition row)
    b64 = np.ascontiguousarray(
        np.tile(np.concatenate([b, b])[None, :], (128, 1))
    ).astype(np.float32)

    in_maps = []
    for i in range(NCORES):
        xf = shard_x(x[i * NPER:(i + 1) * NPER])
        lf = shard_lab(lab[i * NPER:(i + 1) * NPER])
        in_maps.append({
            "x": xf, "lab": lf, "w": W16, "v10": v10, "v08": v08,
            "wv02": wv02, "b64": b64,
        })
    return in_maps


def unshard_output(res_i):
    # out[t, l, c] = raw_sum for node n = t*TN + c; row L = exp-weight sum.
    # Softmax normalization happens here.
    o = np.asarray(res_i["out"]).astype(np.float32)      # [nt, L+1, TN]
    raw = o[:, :L].transpose(0, 2, 1).reshape(NT * TN, L)
    s = o[:, L].reshape(NT * TN)
    return (raw[:NPER] / s[:NPER, None]).astype(np.float32)


def kernel(para_neighbors, para_nei_labels, linear, e_vec, bias):
    from concourse.bass_utils import run_bass_kernel_spmd

    global LAST_RESULT
    with_bias = bool(np.any(np.asarray(bias)))
    key = ("nc", with_bias)
    if key not in _cache:
        _cache[key] = build(with_bias)
        _cache["nc"] = _cache[key]
    nc = _cache[key]

    in_maps = make_in_maps({
        "para_neighbors": para_neighbors, "para_nei_labels": para_nei_labels,
        "linear": linear, "e_vec": e_vec, "bias": bias,
    })
    res = run_bass_kernel_spmd(nc, in_maps, core_ids=list(range(NCORES)))
    LAST_RESULT = res
    outs = [unshard_output(res.results[i]) for i in range(NCORES)]
    return np.ascontiguousarray(np.concatenate(outs, axis=0))


# revision 16
# speedup vs baseline: 1.8702x; 1.0017x over previous
"""Trainium2 Bass kernel: GAT-style attention layer, data-parallel over 8 NeuronCores.

Reference computation (per node n, K=32 neighbors, D=128 features, L=64 labels):
    h     = lrelu(x @ W)                  [N,K,D]
    e     = lrelu(h @ v + bias)           [N,K,1]
    alpha = softmax_k(e)                  [N,K]
    out   = sum_k alpha[n,k] * labels[n,k,:]   [N,L]

Sharding: pure data parallel over nodes (6250/core, zero-padded to 6400).

Structure (v3): every contraction over a 128-partition axis rides the PE with
the LARGE tensor as the *stationary* operand and a 1..4-column moving operand
(PE engine time ~ moving columns only):
  mm1    z^T[e,(k,n)] = W^T @ x^T      16x512-col fp8 matmuls / 256-node tile
  act    h = lrelu(z) (ScalarE chunks, full Prelu) or relu(z) (VectorE
         chunks; the 0.2z part is restored by an x-correction matmul)
  score  s[n, 32s+k]: per (k,sub) a 1-column matmul with the h slice as
         stationary (+ 0.2(Wv)^T x correction for VectorE chunks)
  e/exp  ACT Prelu + Exp on [128,64]
  alphaT 8 32x32 PE transposes place node-quarter j's exp-weights at
         partition block 32j of a persistent (startup-zeroed) PSUM tile;
         one DVE copy -> aT4 [128, 256n] with zeros off-block
  agg    out^T[l, 4 nodes] per matmul: stationary = 4 nodes' label blocks
         stacked [128=(4x32k), 64l] fp8; off-block zeros in aT4 kill the
         cross-node terms. 64 Ldweights+matmuls per tile.
  sums   gpsimd partition-reduce of aT4 -> [1, 256]; softmax 1/sum applied
         on the HOST (sums are DMA'd out, 1KB/tile)

Quantization: x fp8-e3m4, labels fp8-e3m4, weights bf16, out bf16.
Per-core DRAM traffic ~39MB; DMA floor ~4.5us per 256-node tile.
"""
import sys

sys.path.insert(0, "/opt/trn_rl_repo")
import numpy as np

N, K, D, L = 50000, 32, 128, 64
NEG = 0.2
NCORES = 8
NPER = N // NCORES          # 6250
TN = 256                    # nodes per tile
NSUB = TN // 128            # sub-tiles of 128 nodes
NPAD = 6400                 # padded nodes per core
NT = NPAD // TN             # 25 tiles
NCH = 8                     # mm1 chunks per tile (1024 cols each)

# relu-chunk engine schedule: 'A' = ScalarE (full Prelu), 'D' = VectorE
# (relu-only + PE x-correction). Chunk 0 must be 'D' so ACT starts each tile
# with the previous tile's Prelu/Exp.
SCHED = "DADADADA"

LAST_RESULT = None
_cache = {}


def build(with_bias, debug=False):
    import concourse.bass as bass
    import concourse.tile as tile
    from concourse import bacc, mybir

    f32 = mybir.dt.float32
    bf16 = mybir.dt.bfloat16
    f8 = mybir.dt.float8e3
    AF = mybir.ActivationFunctionType
    OP = mybir.AluOpType
    PSUM = bass.MemorySpace.PSUM
    nt = NT

    nc = bacc.Bacc(
        "TRN2", target_bir_lowering=False, debug=False, num_devices=NCORES
    )
    x_ext = nc.declare_dram_parameter("x", [nt, 128, K * TN], f8, False)
    lab_ext = nc.declare_dram_parameter("lab", [nt, 128, 64 * 65], f8, False)
    w_ext = nc.declare_dram_parameter("w", [D, D], bf16, False)
    v10_ext = nc.declare_dram_parameter("v10", [D, 1], bf16, False)
    v08_ext = nc.declare_dram_parameter("v08", [D, 1], bf16, False)
    wv02_ext = nc.declare_dram_parameter("wv02", [D, 1], bf16, False)
    b64_ext = nc.declare_dram_parameter("b64", [128, 64], f32, False)
    out_ext = nc.declare_dram_parameter("out", [nt, L + 1, TN], bf16, isOutput=True)
    if debug:
        sdump_ext = nc.declare_dram_parameter("sdump", [nt, 128, 64], f32, isOutput=True)
        wdump_ext = nc.declare_dram_parameter("wdump", [nt, 128, 64], bf16, isOutput=True)
        hdump_ext = nc.declare_dram_parameter("hdump", [nt, 128, 2048], bf16, isOutput=True)

    with tile.TileContext(nc) as tc:
        with (
            tc.tile_pool(name="const", bufs=1) as const,
            tc.tile_pool(name="xp", bufs=3) as xp,
            tc.tile_pool(name="labp", bufs=2) as labp,
            tc.tile_pool(name="hp", bufs=2) as hp,
            tc.tile_pool(name="smallp", bufs=2) as smallp,
            tc.tile_pool(name="outp", bufs=2) as outp,
            tc.tile_pool(name="zps", bufs=3, space=PSUM) as zps,
            tc.tile_pool(name="smps", bufs=1, space=PSUM) as smps,
        ):
            W_sb = const.tile([128, 128], bf16)
            nc.sync.dma_start(W_sb[:], w_ext[:])
            v10 = const.tile([128, 1], bf16)
            nc.sync.dma_start(v10[:], v10_ext[:])
            v08 = const.tile([128, 1], bf16)
            nc.sync.dma_start(v08[:], v08_ext[:])
            wv02 = const.tile([128, 1], bf16)
            nc.sync.dma_start(wv02[:], wv02_ext[:])
            if with_bias:
                b64 = const.tile([128, 64], f32)
                nc.sync.dma_start(b64[:], b64_ext[:])
            ones = const.tile([128, 128], bf16)
            nc.vector.memset(ones[:], 1.0)
            ident = const.tile([128, 128], bf16)         # identity matrix
            nc.gpsimd.affine_select(
                ident[:], ones[:], pattern=[[1, 128]],
                compare_op=OP.is_equal, fill=0.0, base=0, channel_multiplier=-1,
            )
            # One PSUM bank holds all the small tiles, manually carved:
            # cols 0:64 / 64:128 = s_ps (alternating per tile), 128:384 =
            # o_ps [65,256], 384:512 bitcast bf16 = aT4_ps [128,256].
            smalls = smps.tile([128, 512], f32, name="smalls", tag="smalls")
            s_ps_ab = (smalls[:, 0:64], smalls[:, 64:128])
            o_ps_ap = smalls[0:65, 128:384]
            aT4_ps = smalls[:, 384:512].bitcast(bf16)
            # persistent exp-weight transpose target: node-quarter j occupies
            # partition block 32j; everything off-block is zeroed ONCE here
            # and never written again, so cross-node terms in the batched agg
            # matmul multiply against exact zeros. (memset the underlying f32
            # slice: all-zero bits read back as bf16 zeros. A memset or copy
            # through the bf16 bitcast view fails walrus codegen.)
            nc.vector.memset(smalls[:, 384:512], 0.0)

            # PE warmup burst: dummy matmuls (deps ready ~1us in) while the
            # first x tile loads; the p-state model needs ~3.4us of early PE
            # activity to reach the full 2.4GHz clock.
            warm_ps = zps.tile([128, 1024], f32, name="warm_ps", tag="z")
            for _ in range(32):
                nc.tensor.matmul(
                    warm_ps[:, 0:128], W_sb[:], ones[:], skip_group_check=True
                )

            prev = None   # state of tile t-1 awaiting its score/agg tail

            def emit_head(st):
                """e = lrelu(s) (+bias), w = exp(e). First ACT ops of a tile."""
                ns = st["subs"]
                s_ps = st["s_ps"][:, 0:32 * ns]
                e_sb = smallp.tile([128, 64], f32, name="e_sb", tag="e_sb")
                if with_bias:
                    sb = smallp.tile([128, 64], f32, name="sb", tag="sb")
                    nc.vector.tensor_tensor(sb[:, 0:32 * ns], s_ps, b64[:, 0:32 * ns], OP.add)
                    nc.scalar.activation(e_sb[:, 0:32 * ns], sb[:, 0:32 * ns], AF.Prelu, alpha=NEG)
                else:
                    nc.scalar.activation(e_sb[:, 0:32 * ns], s_ps, AF.Prelu, alpha=NEG)
                w_sb = smallp.tile([128, 64], bf16, name="w_sb", tag="w_sb")
                nc.scalar.activation(w_sb[:, 0:32 * ns], e_sb[:, 0:32 * ns], AF.Exp)
                st["w_sb"] = w_sb
                if debug:
                    sd = smallp.tile([128, 64], f32, name="sd", tag="sd")
                    nc.vector.tensor_copy(sd[:, 0:32 * ns], s_ps)
                    nc.sync.dma_start(sdump_ext[st["t"]][:, 0:32 * ns], sd[:, 0:32 * ns])
                    nc.sync.dma_start(wdump_ext[st["t"]][:, 0:32 * ns], w_sb[:, 0:32 * ns])
                    nc.sync.dma_start(hdump_ext[st["t"]], st["h01"])

            def emit_transp(st):
                """8 32x32 transposes: sub s node-quarter j -> aT4_ps rows
                [32j,32j+32), cols [128s+32j, +32). Then one DVE copy to SBUF
                (zeros off-block come along) + gpsimd per-node sums + DMA."""
                w_sb = st["w_sb"]
                for s in range(st["subs"]):
                    for j in range(4):
                        nc.tensor.transpose(
                            aT4_ps[32 * j:32 * j + 32,
                                    128 * s + 32 * j:128 * s + 32 * j + 32],
                            w_sb[32 * j:32 * j + 32, 32 * s:32 * s + 32],
                            ident[32 * j:32 * j + 32, 32 * j:32 * j + 32],
                            tile_position=(32 * j, 32 * j),
                        )
                aT4 = smallp.tile([128, TN], bf16, name="aT4", tag="aT4")
                nc.vector.tensor_copy(
                    aT4[:, 0:128 * st["subs"]], aT4_ps[:, 0:128 * st["subs"]]
                )
                st["aT4"] = aT4
                st["o_ps"] = o_ps_ap

            def emit_agg(st, q):
                """Aggregation for node groups [16q, 16q+16): one 4-column
                matmul per group of 4 nodes; stationary = their label blocks
                stacked [128, 64] fp8 + a 65th all-ones column whose output
                row is the per-node exp-weight sum (softmax denominator)."""
                lab_sb, aT4, o_ps = st["lab_sb"], st["aT4"], st["o_ps"]
                gmax = 32 * st["subs"]
                for g in range(16 * q, min(16 * q + 16, gmax)):
                    # group g = (s, m): the 4 nodes 128s+32j+m (j=0..3), one
                    # per partition block of aT4 -> moving cols stride 32.
                    # Output columns are contiguous [4g, 4g+4); the host
                    # remaps column 128s+4m+j back to node 128s+32j+m.
                    s_g, m = g // 32, g % 32
                    nc.tensor.matmul(
                        o_ps[:, 4 * g:4 * g + 4],
                        lab_sb[:, 65 * g:65 * g + 65],
                        aT4[:, 128 * s_g + m:128 * s_g + m + 97:32],
                    )

            def emit_out(st):
                w = 128 * st["subs"]
                o_sb = outp.tile([L + 1, TN], bf16, name="o_sb", tag="o_sb")
                nc.scalar.activation(o_sb[:, 0:w], st["o_ps"][:, 0:w], AF.Copy)
                nc.sync.dma_start(out_ext[st["t"]][:, 0:w], o_sb[:, 0:w])

            for t in range(nt):
                x_sb = xp.tile([128, K * TN], f8)
                if t == 0:
                    # quarter the first x load so chunk 0's matmuls start
                    # after ~256KB instead of a full 1MB
                    qn = K * TN // 4
                    for qi in range(4):
                        nc.sync.dma_start(
                            x_sb[:, qi * qn:(qi + 1) * qn],
                            x_ext[t][:, qi * qn:(qi + 1) * qn],
                        )
                else:
                    nc.sync.dma_start(x_sb[:], x_ext[t][:])
                lab_sb = labp.tile([128, 64 * 65], f8)
                nc.sync.dma_start(lab_sb[:], lab_ext[t][:])

                h_sb = hp.tile([128, K * TN], bf16)
                s_ps = s_ps_ab[t % 2]
                tsubs = 1 if t == nt - 1 else NSUB

                def emit_scores(c):
                    # score columns for chunk c's k values (ready once relu c
                    # done). For relu-only (DVE) chunks each column is a
                    # 2-matmul group: h-term then the 0.2(Wv)^T x correction.
                    # Keeping the group members ADJACENT matters: a group
                    # split across other matmuls loses the first term.
                    dve = SCHED[c] == "D"
                    for k in range(4 * c, 4 * c + 4):
                        for s in range(tsubs):
                            col = 32 * s + k
                            base = k * TN + s * 128
                            nc.tensor.matmul(
                                s_ps[:, col:col + 1],
                                h_sb[:, base:base + 128],
                                v08[:] if dve else v10[:],
                                start=True, stop=not dve,
                            )
                            if dve:
                                nc.tensor.matmul(
                                    s_ps[:, col:col + 1],
                                    x_sb[:, base:base + 128], wv02[:],
                                    start=False, stop=True,
                                )

                for c in range(NCH):
                    z_ps = zps.tile([128, 1024], f32, name="z_ps", tag="z")
                    nc.tensor.matmul(
                        z_ps[:, 0:512], W_sb[:], x_sb[:, c * 1024:c * 1024 + 512]
                    )
                    nc.tensor.matmul(
                        z_ps[:, 512:1024], W_sb[:],
                        x_sb[:, c * 1024 + 512:(c + 1) * 1024],
                    )
                    if t == nt - 1:
                        # padding-only sub-tiles: activate the first 128
                        # nodes per k only (the rest is never read)
                        zv = z_ps[:].rearrange("p (k n) -> p k n", k=4)[:, :, 0:128]
                        hv = h_sb[:, c * 1024:(c + 1) * 1024].rearrange(
                            "p (k n) -> p k n", k=4)[:, :, 0:128]
                    else:
                        zv = z_ps[:]
                        hv = h_sb[:, c * 1024:(c + 1) * 1024]
                    if SCHED[c] == "A":
                        nc.scalar.activation(hv, zv, AF.Prelu, alpha=NEG)
                    else:
                        nc.vector.tensor_scalar_max(hv, zv, 0.0)
                    if c == 0 and prev is not None:
                        emit_head(prev)
                    if c == 1 and prev is not None:
                        emit_transp(prev)
                    if c in (2, 3, 4, 5) and prev is not None:
                        emit_agg(prev, c - 2)
                    if c >= 2:
                        emit_scores(c - 2)
                    if c == 6 and prev is not None:
                        emit_out(prev)
                emit_scores(NCH - 2)
                emit_scores(NCH - 1)

                prev = {"t": t, "s_ps": s_ps, "lab_sb": lab_sb, "subs": tsubs,
                        "h01": h_sb[:, 0:2048]}

            # drain the last tile
            emit_head(prev)
            emit_transp(prev)
            for q in range(4):
                emit_agg(prev, q)
            emit_out(prev)
    nc.compile()
    return nc


def shard_x(x, nt=NT, nper=NPER):
    import ml_dtypes

    f8 = ml_dtypes.float8_e3m4
    xs = np.zeros((nt * TN, K, D), f8)
    xs[:nper] = x.astype(f8)
    # [t, n, k, d] -> [t, d, k, n] -> col = k*TN + n
    return np.ascontiguousarray(
        xs.reshape(nt, TN, K, D).transpose(0, 3, 2, 1)
    ).reshape(nt, 128, K * TN)


def shard_lab(lab, nt=NT, nper=NPER):
    import ml_dtypes

    f8 = ml_dtypes.float8_e3m4
    ls = np.zeros((nt * TN, K, L + 1), f8)
    ls[:nper, :, :L] = lab.astype(f8)
    ls[:, :, L] = f8(1.0)   # ones column -> per-node exp-weight sums
    # node n = 128s + 32j + m belongs to group g = 32s + m with quarter j:
    # its labels sit at rows 32j + k of group block [65g, 65g+65)
    l6 = ls.reshape(nt, NSUB, 4, 32, K, L + 1)   # [t, s, j, m, k, l]
    return np.ascontiguousarray(
        l6.transpose(0, 2, 4, 1, 3, 5)           # [t, j, k, s, m, l]
    ).reshape(nt, 128, 64 * 65)


def make_in_maps(inputs):
    import ml_dtypes

    bf16 = ml_dtypes.bfloat16
    x = np.asarray(inputs["para_neighbors"], np.float32)
    lab = np.asarray(inputs["para_nei_labels"], np.float32)
    Wm = np.ascontiguousarray(np.asarray(inputs["linear"], np.float32))
    v = np.ascontiguousarray(np.asarray(inputs["e_vec"], np.float32))
    b = np.asarray(inputs["bias"], np.float32).reshape(K)

    Wb = Wm.astype(bf16).astype(np.float32)
    vb = v.astype(bf16).astype(np.float32)
    W16 = np.ascontiguousarray(Wm.astype(bf16))
    v10 = np.ascontiguousarray(vb.astype(bf16))
    v08 = np.ascontiguousarray((0.8 * vb).astype(bf16))
    # 0.2*(W@v) from the bf16-rounded W/v so the correction matches the PE's z
    wv02 = np.ascontiguousarray((NEG * (Wb @ vb)).astype(bf16))
    # b64[p, 32s+k] = bias[k] (same for every partition row)
    b64 = np.ascontiguousarray(
        np.tile(np.concatenate([b, b])[None, :], (128, 1))
    ).astype(np.float32)

    in_maps = []
    for i in range(NCORES):
        xf = shard_x(x[i * NPER:(i + 1) * NPER])
        lf = shard_lab(lab[i * NPER:(i + 1) * NPER])
        in_maps.append({
            "x": xf, "lab": lf, "w": W16, "v10": v10, "v08": v08,
            "wv02": wv02, "b64": b64,
        })
    return in_maps


def unshard_output(res_i):
    # out[t, l, c]: column c = 128s + 4m + j holds node n = 128s + 32j + m
    # of tile t; row L = exp-weight sum. Softmax normalization happens here.
    o = np.asarray(res_i["out"]).astype(np.float32)      # [nt, L+1, TN]
    c = np.arange(TN)
    node_of_c = 128 * (c // 128) + 32 * (c % 4) + (c % 128) // 4
    inv = np.empty(TN, np.int64)
    inv[node_of_c] = c
    o = o[:, :, inv]                                     # column c' = node c'
    raw = o[:, :L].transpose(0, 2, 1).reshape(NT * TN, L)
    s = o[:, L].reshape(NT * TN)
    return (raw[:NPER] / s[:NPER, None]).astype(np.float32)


def kernel(para_neighbors, para_nei_labels, linear, e_vec, bias):
    from concourse.bass_utils import run_bass_kernel_spmd

    global LAST_RESULT
    with_bias = bool(np.any(np.asarray(bias)))
    key = ("nc", with_bias)
    if key not in _cache:
        _cache[key] = build(with_bias)
        _cache["nc"] = _cache[key]
    nc = _cache[key]

    in_maps = make_in_maps({
        "para_neighbors": para_neighbors, "para_nei_labels": para_nei_labels,
        "linear": linear, "e_vec": e_vec, "bias": bias,
    })
    res = run_bass_kernel_spmd(nc, in_maps, core_ids=list(range(NCORES)))
    LAST_RESULT = res
    outs = [unshard_output(res.results[i]) for i in range(NCORES)]
    return np.ascontiguousarray(np.concatenate(outs, axis=0))
